# revision 11
# baseline (speedup 1.0000x reference)
"""GNN linear-attention kernel for Trainium2 over an axon-tunneled PJRT client.

The device compute for this problem (~2 GFLOP/graph) is trivial next to the
cost of moving data through the tunnel (~30-60 MB/s, ~90 ms/RPC), so the
kernel is organized entirely around the data path:

  - A is a 0/1 adjacency matrix: pack to 1 bit/element on the host
    (134 MB f32 -> 4.2 MB) and unpack on-device.
  - x and the weights ship as one f16 payload per chunk; the output returns
    as f16 and is upcast on the host (rel-err ~3e-4, gate is 2e-2).
  - The batch is split into chunks; uploads, device execution, and downloads
    of different chunks overlap via threads (the tunnel is full-duplex).
  - Everything runs on one NeuronCore: per-op RPC latency dominates any
    benefit of spreading trivial compute across 8 cores.
"""

import os
import time
import numpy as np
from concurrent.futures import ThreadPoolExecutor

_PROF = bool(os.environ.get("KERNEL_PROF"))
_events = []

B, N, D, O = 8, 2048, 128, 128
NCHUNKS = 4
CHUNK = B // NCHUNKS
NW = 3 * D * D + 4 * D  # f16 elements of packed weights per chunk payload

_state = {}


def _get_state():
    if _state:
        return _state
    import jax
    import jax.numpy as jnp

    dev = jax.devices()[0]

    def chunk_compute(bits, xw):
        # bits: (g, N, N//8) u8;  xw: flat f16 = [weights | x chunk]
        i = 0

        def take(n, shape):
            nonlocal i
            t = xw[i : i + n].astype(jnp.float32).reshape(shape)
            i += n
            return t

        W_qk = take(D * D, (D, D))
        W_l = take(D * O, (D, O))
        W_r = take(D * O, (D, O))
        b_qk = take(D, (D,))
        b_l = take(O, (O,))
        W_d = take(D, (1, D))
        b_d = take(D, (D,))
        x = xw[i:].astype(jnp.float32).reshape(CHUNK, N, D)

        shifts = jnp.arange(7, -1, -1, dtype=jnp.uint8)
        A = (bits[..., None] >> shifts) & jnp.uint8(1)
        A = A.reshape(CHUNK, N, N).astype(jnp.float32)
        deg = jnp.sum(A, axis=-1, keepdims=True)
        gate = jax.nn.sigmoid(deg @ W_d + b_d)
        xg = x * gate
        QK = jax.nn.sigmoid(xg @ W_qk + b_qk)
        scores = jnp.einsum("bnd,bmd->bnm", QK, QK) / jnp.sqrt(jnp.float32(D))
        scores = scores * A
        attn = scores / (jnp.sum(scores, axis=-1, keepdims=True) + 1e-6)
        agg = jnp.einsum("bnm,bmd->bnd", attn, xg)
        out = agg @ W_l + b_l + xg @ W_r
        nrm = jnp.linalg.norm(out, axis=-1, keepdims=True)
        out = out / jnp.maximum(nrm, 1e-12)
        # int8 + per-row scale: rows are unit-normalized, so max|v| per row is
        # a tight scale. Halves the bytes on the (slow, uncompressed) down path.
        s = jnp.max(jnp.abs(out), axis=-1, keepdims=True)
        q = jnp.clip(jnp.round(out / s * 127.0), -127, 127).astype(jnp.int8)
        qb = jax.lax.bitcast_convert_type(q, jnp.uint8)
        sb = jax.lax.bitcast_convert_type(s.astype(jnp.float16), jnp.uint8).reshape(
            CHUNK, N, 2
        )
        return jnp.concatenate([qb, sb], axis=-1)  # (C, N, O+2) u8

    _state["jax"] = jax
    _state["dev"] = dev
    _state["fn"] = jax.jit(chunk_compute)
    _state["pool"] = ThreadPoolExecutor(max_workers=8)
    return _state


def _fast_path(x, A, W_qk, b_qk, W_l, b_l, W_r, W_d, b_d):
    st = _get_state()
    jax, dev, fn, pool = st["jax"], st["dev"], st["fn"], st["pool"]

    w16 = np.concatenate(
        [
            np.ascontiguousarray(W_qk, np.float32).reshape(-1),
            np.ascontiguousarray(W_l, np.float32).reshape(-1),
            np.ascontiguousarray(W_r, np.float32).reshape(-1),
            np.ascontiguousarray(b_qk, np.float32).reshape(-1),
            np.ascontiguousarray(b_l, np.float32).reshape(-1),
            np.ascontiguousarray(W_d, np.float32).reshape(-1),
            np.ascontiguousarray(b_d, np.float32).reshape(-1),
        ]
    ).astype(np.float16)
    assert w16.size == NW

    t0 = time.perf_counter()

    def ev(tag, s):
        if _PROF:
            _events.append((tag, s, time.perf_counter() - t0))

    def put(arr, tag=""):
        s = time.perf_counter() - t0
        y = jax.device_put(arr, dev)
        ev("put" + tag, s)
        return y

    # A viewed as bytes: a 0/1 f32 element is nonzero exactly in its top byte,
    # and np.packbits packs any-nonzero as 1, so pack the strided byte view
    # directly (no bool temp).
    Ab = A.view(np.uint8).reshape(B, N, N, 4)

    ys = []
    fetches = []

    def fetch(y, tag=""):
        s = time.perf_counter() - t0
        b = np.asarray(y)  # (g, N, O+2) u8
        ev("fetch" + tag, s)
        q = b[..., :O].view(np.int8).astype(np.float32)
        sc = np.ascontiguousarray(b[..., O : O + 2]).view(np.float16).astype(np.float32)
        return q * (sc * (1.0 / 127.0))

    for c in range(B // CHUNK):
        sl = slice(c * CHUNK, (c + 1) * CHUNK)
        s = time.perf_counter() - t0
        xw = np.concatenate([w16, x[sl].astype(np.float16).reshape(-1)])
        ev(f"xwprep{c}", s)
        xw_fut = pool.submit(put, xw, f"xw{c}")
        s = time.perf_counter() - t0
        bits = np.packbits(Ab[sl, :, :, 3], axis=-1)
        ev(f"pack{c}", s)
        bits_fut = pool.submit(put, bits, f"bits{c}")
        y = fn(bits_fut.result(), xw_fut.result())
        ev(f"disp{c}", time.perf_counter() - t0)
        try:
            y.copy_to_host_async()
        except Exception:
            pass
        ys.append(y)
        fetches.append(pool.submit(fetch, y, str(c)))

    result = np.empty((B, N, O), np.float32)
    for c, f in enumerate(fetches):
        result[c * CHUNK : (c + 1) * CHUNK] = f.result()
    ev("total", 0.0)
    if _PROF:
        for tag, s, e in sorted(_events, key=lambda v: v[2]):
            print(f"  {tag}: {s*1e3:.0f} -> {e*1e3:.0f}")
        _events.clear()
    return result


def _fallback(x, A, W_qk, b_qk, W_l, b_l, W_r, W_d, b_d):
    import jax
    import jax.numpy as jnp

    if "fb" not in _state:

        def f(x_b, A_b, W_qk, b_qk, W_l, b_l, W_r, W_d, b_d):
            deg = jnp.sum(A_b, axis=-1, keepdims=True)
            gate = jax.nn.sigmoid(deg @ W_d + b_d)
            xg = x_b * gate
            QK = jax.nn.sigmoid(xg @ W_qk + b_qk)
            scores = (QK @ QK.T) / jnp.sqrt(jnp.float32(D))
            scores = scores * A_b
            attn = scores / (jnp.sum(scores, axis=-1, keepdims=True) + 1e-6)
            agg = attn @ xg
            out = agg @ W_l + b_l + xg @ W_r
            nrm = jnp.linalg.norm(out, axis=-1, keepdims=True)
            return out / jnp.maximum(nrm, 1e-12)

        _state["fb"] = jax.jit(f)
    fn = _state["fb"]
    dev = jax.devices()[0]
    ws = [jax.device_put(np.asarray(t), dev) for t in (W_qk, b_qk, W_l, b_l, W_r, W_d, b_d)]
    out = np.stack(
        [np.asarray(fn(jax.device_put(x[b], dev), jax.device_put(A[b], dev), *ws)) for b in range(B)]
    )
    return out.astype(np.float32)


def kernel(x, A, W_qk, b_qk, W_l, b_l, W_r, W_d, b_d):
    x = np.ascontiguousarray(x, np.float32)
    A = np.ascontiguousarray(A, np.float32)
    try:
        return _fast_path(x, A, W_qk, b_qk, W_l, b_l, W_r, W_d, b_d)
    except Exception:
        return _fallback(x, A, W_qk, b_qk, W_l, b_l, W_r, W_d, b_d)


# revision 13
# speedup vs baseline: 18.3020x; 18.3020x over previous
"""GNN linear-attention kernel for Trainium2 over an axon-tunneled PJRT client.

The device compute for this problem (~2 GFLOP/graph) is trivial next to the
cost of moving data through the tunnel (~30-60 MB/s, ~90 ms/RPC), so the
kernel is organized entirely around the data path:

  - A is a 0/1 adjacency matrix: pack to 1 bit/element on the host
    (134 MB f32 -> 4.2 MB) and unpack on-device.
  - x and the weights ship as one f16 payload per chunk; the output returns
    as f16 and is upcast on the host (rel-err ~3e-4, gate is 2e-2).
  - The batch is split into chunks; uploads, device execution, and downloads
    of different chunks overlap via threads (the tunnel is full-duplex).
  - Everything runs on one NeuronCore: per-op RPC latency dominates any
    benefit of spreading trivial compute across 8 cores.
"""

import os
import time
import numpy as np
from concurrent.futures import ThreadPoolExecutor

_PROF = bool(os.environ.get("KERNEL_PROF"))
_events = []

B, N, D, O = 8, 2048, 128, 128
NCHUNKS = 4
CHUNK = B // NCHUNKS
NW = 3 * D * D + 4 * D  # f16 elements of packed weights per chunk payload

_state = {}


def _get_state():
    if _state:
        return _state
    import jax
    import jax.numpy as jnp

    dev = jax.devices()[0]

    def chunk_compute(bits, xw):
        # bits: (g, N, N//8) u8;  xw: flat f16 = [weights | x chunk]
        i = 0

        def take(n, shape):
            nonlocal i
            t = xw[i : i + n].astype(jnp.float32).reshape(shape)
            i += n
            return t

        W_qk = take(D * D, (D, D))
        W_l = take(D * O, (D, O))
        W_r = take(D * O, (D, O))
        b_qk = take(D, (D,))
        b_l = take(O, (O,))
        W_d = take(D, (1, D))
        b_d = take(D, (D,))
        x = xw[i:].astype(jnp.float32).reshape(CHUNK, N, D)

        shifts = jnp.arange(7, -1, -1, dtype=jnp.uint8)
        A = (bits[..., None] >> shifts) & jnp.uint8(1)
        A = A.reshape(CHUNK, N, N).astype(jnp.float32)
        deg = jnp.sum(A, axis=-1, keepdims=True)
        gate = jax.nn.sigmoid(deg @ W_d + b_d)
        xg = x * gate
        QK = jax.nn.sigmoid(xg @ W_qk + b_qk)
        scores = jnp.einsum("bnd,bmd->bnm", QK, QK) / jnp.sqrt(jnp.float32(D))
        scores = scores * A
        attn = scores / (jnp.sum(scores, axis=-1, keepdims=True) + 1e-6)
        agg = jnp.einsum("bnm,bmd->bnd", attn, xg)
        out = agg @ W_l + b_l + xg @ W_r
        nrm = jnp.linalg.norm(out, axis=-1, keepdims=True)
        out = out / jnp.maximum(nrm, 1e-12)
        # 8-bit codes + per-row scale: rows are unit-normalized, so max|v| per
        # row is a tight scale. Halves the bytes on the (slow, uncompressed)
        # down path. All-arithmetic encode: neuronx-cc crashes on bitcasts.
        s = jnp.max(jnp.abs(out), axis=-1, keepdims=True)
        qb = (jnp.clip(jnp.round(out / s * 127.0), -127, 127) + 128.0).astype(
            jnp.uint8
        )
        sq = jnp.round(s * 65535.0).astype(jnp.int32)
        shi = (sq >> 8).astype(jnp.uint8)
        slo = (sq & 0xFF).astype(jnp.uint8)
        return jnp.concatenate([qb, shi, slo], axis=-1)  # (C, N, O+2) u8

    _state["jax"] = jax
    _state["dev"] = dev
    _state["fn"] = jax.jit(chunk_compute)
    _state["pool"] = ThreadPoolExecutor(max_workers=8)
    return _state


def _fast_path(x, A, W_qk, b_qk, W_l, b_l, W_r, W_d, b_d):
    st = _get_state()
    jax, dev, fn, pool = st["jax"], st["dev"], st["fn"], st["pool"]

    w16 = np.concatenate(
        [
            np.ascontiguousarray(W_qk, np.float32).reshape(-1),
            np.ascontiguousarray(W_l, np.float32).reshape(-1),
            np.ascontiguousarray(W_r, np.float32).reshape(-1),
            np.ascontiguousarray(b_qk, np.float32).reshape(-1),
            np.ascontiguousarray(b_l, np.float32).reshape(-1),
            np.ascontiguousarray(W_d, np.float32).reshape(-1),
            np.ascontiguousarray(b_d, np.float32).reshape(-1),
        ]
    ).astype(np.float16)
    assert w16.size == NW

    t0 = time.perf_counter()

    def ev(tag, s):
        if _PROF:
            _events.append((tag, s, time.perf_counter() - t0))

    def put(arr, tag=""):
        s = time.perf_counter() - t0
        y = jax.device_put(arr, dev)
        ev("put" + tag, s)
        return y

    # A viewed as bytes: a 0/1 f32 element is nonzero exactly in its top byte,
    # and np.packbits packs any-nonzero as 1, so pack the strided byte view
    # directly (no bool temp).
    Ab = A.view(np.uint8).reshape(B, N, N, 4)

    ys = []
    fetches = []

    def fetch(y, tag=""):
        s = time.perf_counter() - t0
        b = np.asarray(y)  # (g, N, O+2) u8
        ev("fetch" + tag, s)
        q = b[..., :O].astype(np.float32) - 128.0
        sc = (
            b[..., O].astype(np.float32) * 256.0 + b[..., O + 1].astype(np.float32)
        ) * (1.0 / 65535.0)
        return q * (sc[..., None] * (1.0 / 127.0))

    for c in range(B // CHUNK):
        sl = slice(c * CHUNK, (c + 1) * CHUNK)
        s = time.perf_counter() - t0
        xw = np.concatenate([w16, x[sl].astype(np.float16).reshape(-1)])
        ev(f"xwprep{c}", s)
        xw_fut = pool.submit(put, xw, f"xw{c}")
        s = time.perf_counter() - t0
        bits = np.packbits(Ab[sl, :, :, 3], axis=-1)
        ev(f"pack{c}", s)
        bits_fut = pool.submit(put, bits, f"bits{c}")
        y = fn(bits_fut.result(), xw_fut.result())
        ev(f"disp{c}", time.perf_counter() - t0)
        try:
            y.copy_to_host_async()
        except Exception:
            pass
        ys.append(y)
        fetches.append(pool.submit(fetch, y, str(c)))

    result = np.empty((B, N, O), np.float32)
    for c, f in enumerate(fetches):
        result[c * CHUNK : (c + 1) * CHUNK] = f.result()
    ev("total", 0.0)
    if _PROF:
        for tag, s, e in sorted(_events, key=lambda v: v[2]):
            print(f"  {tag}: {s*1e3:.0f} -> {e*1e3:.0f}")
        _events.clear()
    return result


def _fallback(x, A, W_qk, b_qk, W_l, b_l, W_r, W_d, b_d):
    import jax
    import jax.numpy as jnp

    if "fb" not in _state:

        def f(x_b, A_b, W_qk, b_qk, W_l, b_l, W_r, W_d, b_d):
            deg = jnp.sum(A_b, axis=-1, keepdims=True)
            gate = jax.nn.sigmoid(deg @ W_d + b_d)
            xg = x_b * gate
            QK = jax.nn.sigmoid(xg @ W_qk + b_qk)
            scores = (QK @ QK.T) / jnp.sqrt(jnp.float32(D))
            scores = scores * A_b
            attn = scores / (jnp.sum(scores, axis=-1, keepdims=True) + 1e-6)
            agg = attn @ xg
            out = agg @ W_l + b_l + xg @ W_r
            nrm = jnp.linalg.norm(out, axis=-1, keepdims=True)
            return out / jnp.maximum(nrm, 1e-12)

        _state["fb"] = jax.jit(f)
    fn = _state["fb"]
    dev = jax.devices()[0]
    ws = [jax.device_put(np.asarray(t), dev) for t in (W_qk, b_qk, W_l, b_l, W_r, W_d, b_d)]
    out = np.stack(
        [np.asarray(fn(jax.device_put(x[b], dev), jax.device_put(A[b], dev), *ws)) for b in range(B)]
    )
    return out.astype(np.float32)


def kernel(x, A, W_qk, b_qk, W_l, b_l, W_r, W_d, b_d):
    x = np.ascontiguousarray(x, np.float32)
    A = np.ascontiguousarray(A, np.float32)
    try:
        return _fast_path(x, A, W_qk, b_qk, W_l, b_l, W_r, W_d, b_d)
    except Exception:
        return _fallback(x, A, W_qk, b_qk, W_l, b_l, W_r, W_d, b_d)


# revision 14
# speedup vs baseline: 19.1082x; 1.0441x over previous
"""GNN linear-attention kernel for Trainium2 over an axon-tunneled PJRT client.

The device compute for this problem (~2 GFLOP/graph) is trivial next to the
cost of moving data through the tunnel (~25-60 MB/s, ~90 ms/RPC), so the
kernel is organized entirely around the data path:

  - A is a 0/1 adjacency matrix: packed to 1 bit/element on the host
    (134 MB f32 -> 4.2 MB) and unpacked on-device.
  - x ships as 12-bit fixed point (3.1 MB), weights as f16 once (with the
    first chunk); the output returns as 8-bit codes + per-row scale and is
    dequantized on the host (total rel-err ~7e-3, gate is 2e-2).
  - The batch is split into chunks; uploads, device execution, and downloads
    of different chunks overlap (threads; the tunnel is ~half-duplex).
  - Everything runs on one NeuronCore: per-op RPC latency dominates any
    benefit of spreading the (trivial) compute across 8 cores.
"""

import os
import time
import numpy as np
from concurrent.futures import ThreadPoolExecutor

B, N, D, O = 8, 2048, 128, 128
NCHUNKS = 4
CHUNK = B // NCHUNKS
NW = 3 * D * D + 4 * D + 1  # f16 payload: weights + x scale

_PROF = bool(os.environ.get("KERNEL_PROF"))
_events = []
_state = {}


def _get_state():
    if _state:
        return _state
    import jax
    import jax.numpy as jnp

    dev = jax.devices()[0]

    def unpack_x(x3, xsc):
        # x3: flat u8, three planes of 12-bit codes (b0|b1|b2); xsc: f16 scalar
        m = CHUNK * N * D // 2
        b0 = x3[0 * m : 1 * m].astype(jnp.int32)
        b1 = x3[1 * m : 2 * m].astype(jnp.int32)
        b2 = x3[2 * m : 3 * m].astype(jnp.int32)
        u0 = b0 | ((b1 & 0xF) << 8)
        u1 = (b1 >> 4) | (b2 << 4)
        q = jnp.stack([u0, u1], axis=-1).reshape(CHUNK, N, D)
        return (q - 2048).astype(jnp.float32) * (
            xsc.astype(jnp.float32) / 2047.0
        )

    def compute(bits, x, W_qk, b_qk, W_l, b_l, W_r, W_d, b_d):
        shifts = jnp.arange(7, -1, -1, dtype=jnp.uint8)
        A = (bits[..., None] >> shifts) & jnp.uint8(1)
        A = A.reshape(CHUNK, N, N).astype(jnp.float32)
        deg = jnp.sum(A, axis=-1, keepdims=True)
        gate = jax.nn.sigmoid(deg @ W_d + b_d)
        xg = x * gate
        QK = jax.nn.sigmoid(xg @ W_qk + b_qk)
        scores = jnp.einsum("bnd,bmd->bnm", QK, QK) / jnp.sqrt(jnp.float32(D))
        scores = scores * A
        attn = scores / (jnp.sum(scores, axis=-1, keepdims=True) + 1e-6)
        agg = jnp.einsum("bnm,bmd->bnd", attn, xg)
        out = agg @ W_l + b_l + xg @ W_r
        nrm = jnp.linalg.norm(out, axis=-1, keepdims=True)
        out = out / jnp.maximum(nrm, 1e-12)
        # 8-bit codes + per-row scale: rows are unit-normalized, so max|v| per
        # row is a tight scale. Halves the bytes on the (slow, uncompressed)
        # down path. All-arithmetic encode: neuronx-cc crashes on bitcasts.
        s = jnp.max(jnp.abs(out), axis=-1, keepdims=True)
        qb = (jnp.clip(jnp.round(out / s * 127.0), -127, 127) + 128.0).astype(
            jnp.uint8
        )
        sq = jnp.round(s * 65535.0).astype(jnp.int32)
        shi = (sq >> 8).astype(jnp.uint8)
        slo = (sq & 0xFF).astype(jnp.uint8)
        return jnp.concatenate([qb, shi, slo], axis=-1)  # (C, N, O+2) u8

    def parse_w(w16):
        i = 0

        def take(n, shape):
            nonlocal i
            t = w16[i : i + n].astype(jnp.float32).reshape(shape)
            i += n
            return t

        W_qk = take(D * D, (D, D))
        W_l = take(D * O, (D, O))
        W_r = take(D * O, (D, O))
        b_qk = take(D, (D,))
        b_l = take(O, (O,))
        W_d = take(D, (1, D))
        b_d = take(D, (D,))
        xsc = w16[i]
        return (W_qk, b_qk, W_l, b_l, W_r, W_d, b_d), xsc

    def fn_first(bits, x3, w16):
        ws, xsc = parse_w(w16)
        x = unpack_x(x3, xsc)
        return (compute(bits, x, *ws), xsc) + ws

    def fn_rest(bits, x3, xsc, *ws):
        x = unpack_x(x3, xsc)
        return compute(bits, x, *ws)

    _state["jax"] = jax
    _state["dev"] = dev
    _state["fn_first"] = jax.jit(fn_first)
    _state["fn_rest"] = jax.jit(fn_rest)
    _state["pool"] = ThreadPoolExecutor(max_workers=10)
    return _state


def _pack_x12(xc, inv_step):
    # xc: (CHUNK, N, D) f32 -> flat u8 of three 12-bit planes
    q = (np.round(xc.reshape(-1) * inv_step) + 2048.0).astype(np.uint16)
    q0 = q[0::2]
    q1 = q[1::2]
    b0 = (q0 & 0xFF).astype(np.uint8)
    b1 = ((q0 >> 8) | ((q1 & 0xF) << 4)).astype(np.uint8)
    b2 = (q1 >> 4).astype(np.uint8)
    return np.concatenate([b0, b1, b2])


def _fast_path(x, A, W_qk, b_qk, W_l, b_l, W_r, W_d, b_d):
    st = _get_state()
    jax, dev, pool = st["jax"], st["dev"], st["pool"]
    fn_first, fn_rest = st["fn_first"], st["fn_rest"]

    xsc = float(np.abs(x).max())
    inv_step = 2047.0 / xsc if xsc > 0 else 0.0
    w16 = np.concatenate(
        [
            np.ascontiguousarray(W_qk, np.float32).reshape(-1),
            np.ascontiguousarray(W_l, np.float32).reshape(-1),
            np.ascontiguousarray(W_r, np.float32).reshape(-1),
            np.ascontiguousarray(b_qk, np.float32).reshape(-1),
            np.ascontiguousarray(b_l, np.float32).reshape(-1),
            np.ascontiguousarray(W_d, np.float32).reshape(-1),
            np.ascontiguousarray(b_d, np.float32).reshape(-1),
            np.float32([xsc]),
        ]
    ).astype(np.float16)
    assert w16.size == NW

    t0 = time.perf_counter()

    def ev(tag, s):
        if _PROF:
            _events.append((tag, s, time.perf_counter() - t0))

    def put(arr, tag=""):
        s = time.perf_counter() - t0
        y = jax.device_put(arr, dev)
        ev("put" + tag, s)
        return y

    # A viewed as bytes: a 0/1 f32 element is nonzero exactly in its top byte,
    # and np.packbits packs any-nonzero as 1, so pack the strided byte view
    # directly (no bool temp).
    Ab = A.view(np.uint8).reshape(B, N, N, 4)

    def fetch(y, tag=""):
        s = time.perf_counter() - t0
        b = np.asarray(y)  # (g, N, O+2) u8
        ev("fetch" + tag, s)
        q = b[..., :O].astype(np.float32) - 128.0
        sc = (
            b[..., O].astype(np.float32) * 256.0 + b[..., O + 1].astype(np.float32)
        ) * (1.0 / 65535.0)
        return q * (sc[..., None] * (1.0 / 127.0))

    fetches = []
    wdevs = None
    for c in range(NCHUNKS):
        sl = slice(c * CHUNK, (c + 1) * CHUNK)
        s = time.perf_counter() - t0
        x3 = _pack_x12(x[sl], inv_step)
        ev(f"xprep{c}", s)
        x3_fut = pool.submit(put, x3, f"x{c}")
        if c == 0:
            w_fut = pool.submit(put, w16, "w")
        s = time.perf_counter() - t0
        bits = np.packbits(Ab[sl, :, :, 3], axis=-1)
        ev(f"pack{c}", s)
        bits_fut = pool.submit(put, bits, f"bits{c}")
        if c == 0:
            res = fn_first(bits_fut.result(), x3_fut.result(), w_fut.result())
            y, wdevs = res[0], res[1:]
        else:
            y = fn_rest(bits_fut.result(), x3_fut.result(), *wdevs)
        ev(f"disp{c}", time.perf_counter() - t0)
        try:
            y.copy_to_host_async()
        except Exception:
            pass
        fetches.append(pool.submit(fetch, y, str(c)))

    result = np.empty((B, N, O), np.float32)
    for c, f in enumerate(fetches):
        result[c * CHUNK : (c + 1) * CHUNK] = f.result()
    ev("total", 0.0)
    if _PROF:
        for tag, s, e in sorted(_events, key=lambda v: v[2]):
            print(f"  {tag}: {s*1e3:.0f} -> {e*1e3:.0f}")
        _events.clear()
    return result


def _fallback(x, A, W_qk, b_qk, W_l, b_l, W_r, W_d, b_d):
    import jax
    import jax.numpy as jnp

    if "fb" not in _state:

        def f(x_b, A_b, W_qk, b_qk, W_l, b_l, W_r, W_d, b_d):
            deg = jnp.sum(A_b, axis=-1, keepdims=True)
            gate = jax.nn.sigmoid(deg @ W_d + b_d)
            xg = x_b * gate
            QK = jax.nn.sigmoid(xg @ W_qk + b_qk)
            scores = (QK @ QK.T) / jnp.sqrt(jnp.float32(D))
            scores = scores * A_b
            attn = scores / (jnp.sum(scores, axis=-1, keepdims=True) + 1e-6)
            agg = attn @ xg
            out = agg @ W_l + b_l + xg @ W_r
            nrm = jnp.linalg.norm(out, axis=-1, keepdims=True)
            return out / jnp.maximum(nrm, 1e-12)

        _state["fb"] = jax.jit(f)
    fn = _state["fb"]
    dev = jax.devices()[0]
    ws = [jax.device_put(np.asarray(t), dev) for t in (W_qk, b_qk, W_l, b_l, W_r, W_d, b_d)]
    out = np.stack(
        [np.asarray(fn(jax.device_put(x[b], dev), jax.device_put(A[b], dev), *ws)) for b in range(B)]
    )
    return out.astype(np.float32)


def kernel(x, A, W_qk, b_qk, W_l, b_l, W_r, W_d, b_d):
    x = np.ascontiguousarray(x, np.float32)
    A = np.ascontiguousarray(A, np.float32)
    try:
        return _fast_path(x, A, W_qk, b_qk, W_l, b_l, W_r, W_d, b_d)
    except Exception:
        return _fallback(x, A, W_qk, b_qk, W_l, b_l, W_r, W_d, b_d)
